# revision 21
# baseline (speedup 1.0000x reference)
"""TRN2 Bass kernel for nn_AttentionModuleV1 (gnn_message_passing).

Math note: the reference broadcasts features to a neighbor axis L=16 where
every slice is identical, so softmax over L is exactly uniform (1/16) and
the module collapses to (per row n of the N=16384 point axis):

    pos = relu(features  @ Wk.T)
    h   = relu(bn1(features2 @ Wv1.T))
    val = relu(bn2(h @ Wv2.T))
    vc  = sigmoid(pos @ Wv_coef.T)
    out = val + pos * vc

(xyz, Wa, Wq_coef, Wk_coef, Wqk_coef do not affect the output: they only
feed the softmax logits, which are constant along L.)

Sharding: pure data parallel over N across 8 cores (2048 rows each).

Implementation (fast path, used when the folded bn biases are exactly 0 -
which setup_inputs' identity bn params always produce): a hand-scheduled
Bass program with MANUAL semaphore synchronization instead of TileContext.
The Tile framework allocated ~254 semaphores and its epilogue cleared them
one instruction at a time (~8us of teardown storm), plus serialized all 13
DMA issues on the sync queue (~600ns each).  The manual program uses 10
counting semaphores, attaches waits directly to consuming instructions,
issues the weight DMA from the ACT queue in parallel with x-tile DMAs on
the sync queue, and lets SP issue output DMAs so the DVE stream stays
dense.  Engine split per 512-row tile (NTILE=512, 4 tiles/core):
  PE  16 matmuls (bf16, ~3.4us)          <- steady-state bottleneck
  ACT h-relu, vc-sigmoid, val-relu drains as [128,1024] 2-bank reads
  DVE pos-relu drain, prod=pos*vc, out=val+prod (bf16)
A PE warmup burst on zeroed scratch bridges the initial DMA wait so the
p-state ramp is done when real matmuls start.  Teardown: 2 barriers + 10
semaphore clears (~1us).

The general path (nonzero bn biases) keeps the original TileContext
implementation - correct for arbitrary inputs, just slower.
"""
import sys

sys.path.insert(0, "/opt/trn_rl_repo")

import numpy as np
import ml_dtypes
from concourse import bacc, mybir
import concourse.tile as tile
from concourse.bass_utils import run_bass_kernel_spmd
from concourse.alu_op_type import AluOpType

F32 = mybir.dt.float32
BF16 = mybir.dt.bfloat16
AF = mybir.ActivationFunctionType
NPBF16 = ml_dtypes.bfloat16

N_TOTAL = 16384
C = 256        # input feature channels
OUT = 256      # output channels
NCORES = 8
NSH = N_TOTAL // NCORES   # 2048 rows per core
P = 128
KC = C // P    # contraction chunks
OC = OUT // P  # output-channel chunks
NTILE = 512    # n-columns per pipeline tile (1 PSUM bank per acc half)
NT = NSH // NTILE
BN_EPS = 1e-5
NWARM_LONG = 7    # [128,512] warmup matmuls bridging the head DMA wait
NWARM_SHORT = 6   # [128,128] fine-grained handoff warmups

_cache = {}


# --------------------------------------------------------------------------
# Fast path: manual-semaphore program (requires folded bn biases == 0)
# --------------------------------------------------------------------------

def _build_manual():
    nc = bacc.Bacc(None, target_bir_lowering=False, debug=True)

    x_d = nc.declare_dram_parameter("xsw", [NT, P, 2 * KC * NTILE], BF16,
                                    isOutput=False)
    w_d = nc.declare_dram_parameter("wsw", [P, 4 * KC * OUT], BF16,
                                    isOutput=False)
    out_d = nc.declare_dram_parameter("osw", [NT, P, OC * NTILE], BF16,
                                      isOutput=True)

    pe, act, vec, sp, gp = nc.tensor, nc.scalar, nc.vector, nc.sync, nc.gpsimd

    # ---- SBUF / PSUM allocation (manual; partition dim first) -----------
    xs = nc.alloc_sbuf_tensor("xs", [P, NT, 2, KC, NTILE], BF16)
    w = nc.alloc_sbuf_tensor("w", [P, 4, KC, OUT], BF16)
    posb = nc.alloc_sbuf_tensor("posb", [P, 2, OC * NTILE], BF16)
    hb = nc.alloc_sbuf_tensor("hb", [P, 2, OC * NTILE], BF16)
    vcb = nc.alloc_sbuf_tensor("vcb", [P, 2, OC * NTILE], BF16)
    valb = nc.alloc_sbuf_tensor("valb", [P, 2, OC * NTILE], BF16)
    prodb = nc.alloc_sbuf_tensor("prodb", [P, OC * NTILE], BF16)
    outt = nc.alloc_sbuf_tensor("outt", [P, 2, OC * NTILE], BF16)
    scratch = nc.alloc_sbuf_tensor("scratch", [P, NTILE], BF16)
    dumm = nc.alloc_sbuf_tensor("dumm", [P, 1], F32)

    acc_pos = nc.alloc_psum_tensor("acc_pos", [P, OC * NTILE], F32)
    acc_h = nc.alloc_psum_tensor("acc_h", [P, OC * NTILE], F32)
    acc_vc = nc.alloc_psum_tensor("acc_vc", [P, OC * NTILE], F32)
    acc_val = nc.alloc_psum_tensor("acc_val", [P, OC * NTILE], F32)

    # ---- semaphores (one per DMA; compute sems count with >= waits).
    # All allocated at S[207+]: the compiler appends a fixed epilogue where
    # each engine wipes a fixed slice of the semaphore space (PE: S[3..53],
    # ..., SP: S[207..255]) as soon as it runs off the end of its program.
    # Keeping every kernel semaphore in SP's slice makes SP the only wiper
    # of live state, and SP's final DMA-completion waits order the wipe
    # after all uses - so no end-of-kernel barrier or clears are needed.
    _nsem = [207]

    def _sem(name):
        h = nc.alloc_semaphore(name, num=_nsem[0])
        _nsem[0] += 1
        return h

    s_x = [[_sem(f"s_x{t}{s}") for s in range(2)] for t in range(NT)]
    s_wk = _sem("s_wk")
    s_w1 = _sem("s_w1")
    s_wr = _sem("s_wr")
    s_ws = _sem("s_ws")      # warmup scratch memset done
    s_pe = _sem("s_pe")      # PE oc-half group completions
    s_pos = _sem("s_pos")    # DVE pos drains (1/tile)
    s_h = _sem("s_h")        # h drains (2/tile: ACT+DVE)
    s_vc = _sem("s_vc")      # vc drains (t0,t1: 1; t2,t3: 2)
    s_val = _sem("s_val")    # val drains (same counting)
    s_dve = _sem("s_dve")    # DVE prod/add completions
    s_od = [_sem(f"s_od{i}") for i in range(NT + 1)]

    # cumulative drain counts per tile (tiles 2,3 drain vc/val per-oc)
    NVC = {0: 1, 1: 2, 2: 4, 3: 6}

    def pre_waits(eng, waits):
        # an instruction carries at most one attached wait; emit extras as
        # standalone event-sem waits before it, return the one to attach
        for sem, val in waits[:-1]:
            eng.wait_ge(sem, val)
        return waits[-1] if waits else None

    # ---- gpsimd: zero the warmup scratch --------------------------------
    gp.memset(scratch.ap(), 0.0).then_inc(s_ws)

    # ---- SP queue, priority order: wk, x0 halves, w1, remaining x ------
    wv = w_d.ap().rearrange("p (j kc o) -> p j kc o", j=4, kc=KC)
    xv = [x_d.ap()[t].rearrange("p (s kc n) -> p s kc n", s=2, kc=KC)
          for t in range(NT)]

    def load_x(t, s):
        sp.dma_start(out=xs[:, t, s], in_=xv[t][:, s]).then_inc(
            s_x[t][s], 16)

    sp.dma_start(out=w[:, 0], in_=wv[:, 0]).then_inc(s_wk, 16)
    load_x(0, 0)
    load_x(0, 1)
    sp.dma_start(out=w[:, 1], in_=wv[:, 1]).then_inc(s_w1, 16)
    for t in range(1, NT):
        load_x(t, 0)
        load_x(t, 1)

    # ---- ACT: dummy sigmoid hoists the table load into the DMA ramp;
    # the late weights (wvc/wv2, first needed by V(0)) ride the ACT queue
    act.activation(dumm.ap(), nc.const_aps.aps[(F32, 0.0)], AF.Sigmoid)
    act.dma_start(out=w[:, 2:4], in_=wv[:, 2:4]).then_inc(s_wr, 16)

    # ---- PE program ------------------------------------------------------
    # warmup burst (p-state ramp while DMAs land); one accumulation group
    job_counter = [0]
    job_idx = {}

    for i in range(NWARM_LONG):
        inst = pe.matmul(acc_val[:, 0:NTILE], scratch[:, :P], scratch[:, :],
                         start=(i == 0), stop=False)
        if i == 0:
            inst._wait_ge(s_ws, 1)
    for i in range(NWARM_SHORT):
        inst = pe.matmul(acc_val[:, 0:P], scratch[:, :P], scratch[:, :P],
                         start=False, stop=(i == NWARM_SHORT - 1))
    inst.then_inc(s_pe)
    job_counter[0] += 1
    warm_idx = job_counter[0]

    def job(kind, t, acc, op, rhs_kc, waits):
        # one acc group: OC x KC matmuls; waits precede the 1st matmul,
        # s_pe increments on each oc-half's final matmul
        attach = pre_waits(pe, waits)
        for oc in range(OC):
            for kc in range(KC):
                inst = pe.matmul(
                    acc[:, oc * NTILE:(oc + 1) * NTILE],
                    w[:, op, kc, oc * P:(oc + 1) * P],
                    rhs_kc(kc),
                    start=(kc == 0), stop=(kc == KC - 1))
                if oc == 0 and kc == 0 and attach is not None:
                    inst._wait_ge(*attach)
                if kc == KC - 1:
                    inst.then_inc(s_pe)
                    job_counter[0] += 1
                    job_idx[(kind, t, oc)] = job_counter[0]

    def head_jobs(t, which):
        if which == "P":
            waits = [(s_x[t][0], 16)]
            if t == 0:
                waits.append((s_wk, 16))
            if t >= 1:
                waits.append((s_pos, t))
            job("P", t, acc_pos, 0, lambda kc: xs[:, t, 0, kc, :], waits)
        else:
            waits = [(s_x[t][1], 16)]
            if t == 0:
                waits.append((s_w1, 16))
            if t >= 1:
                waits.append((s_h, 2 * t))
            job("H", t, acc_h, 1, lambda kc: xs[:, t, 1, kc, :], waits)

    def tail_jobs(tp, which):
        r = tp % 2
        if which == "V":
            waits = [(s_pos, tp + 1)]
            if tp == 0:
                waits.append((s_wr, 16))
            else:
                waits.append((s_vc, NVC[tp - 1]))
            job("V", tp, acc_vc, 2,
                lambda kc: posb[:, r, kc * NTILE:(kc + 1) * NTILE], waits)
        else:
            waits = [(s_h, 2 * (tp + 1))]
            if tp == 0:
                waits.append((s_pe, warm_idx))
            else:
                waits.append((s_val, NVC[tp - 1]))
            job("W", tp, acc_val, 3,
                lambda kc: hb[:, r, kc * NTILE:(kc + 1) * NTILE], waits)

    # PE order: V(0) runs early (between H(0) and P(1)) and W(0) after
    # H(1), giving the x1 loads extra DMA pipeline slack while only
    # exposing the tile-0 pos/h drain latencies once; tiles 1+ run V/W a
    # full tile behind (lag-1) so every later drain is latency-hidden.
    pj, hj, vj, wj = (lambda t: head_jobs(t, "P")), (lambda t: head_jobs(
        t, "H")), (lambda t: tail_jobs(t, "V")), (lambda t: tail_jobs(
            t, "W"))
    pj(0); hj(0); vj(0); pj(1); hj(1); wj(0)
    pj(2); hj(2); vj(1); wj(1)
    pj(3); hj(3); vj(2); wj(2)
    vj(3); wj(3)

    def act_h0(t):
        inst = act.activation(hb[:, t % 2, 0:NTILE], acc_h[:, 0:NTILE],
                              AF.Relu)
        inst._wait_ge(s_pe, job_idx[("H", t, 0)])
        inst.then_inc(s_h)

    def act_vc(tp):
        r = tp % 2
        if tp < 2:
            inst = act.activation(vcb[:, r], acc_vc.ap(), AF.Sigmoid)
            inst._wait_ge(s_pe, job_idx[("V", tp, 1)])
            inst.then_inc(s_vc)
        else:
            for oc in range(OC):
                sl = slice(oc * NTILE, (oc + 1) * NTILE)
                waits = [(s_pe, job_idx[("V", tp, oc)])]
                if oc == 0:
                    waits.append((s_dve, 2 * (tp - 2) + 1))
                attach = pre_waits(act, waits)
                inst = act.activation(vcb[:, r, sl], acc_vc[:, sl],
                                      AF.Sigmoid)
                inst._wait_ge(*attach)
                inst.then_inc(s_vc)

    def act_val(tp):
        r = tp % 2
        if tp < 2:
            inst = act.activation(valb[:, r], acc_val.ap(), AF.Relu)
            inst._wait_ge(s_pe, job_idx[("W", tp, 1)])
            inst.then_inc(s_val)
        else:
            for oc in range(OC):
                sl = slice(oc * NTILE, (oc + 1) * NTILE)
                waits = [(s_pe, job_idx[("W", tp, oc)])]
                if oc == 0:
                    waits.append((s_dve, 2 * (tp - 2) + 2))
                attach = pre_waits(act, waits)
                inst = act.activation(valb[:, r, sl], acc_val[:, sl],
                                      AF.Relu)
                inst._wait_ge(*attach)
                inst.then_inc(s_val)

    def dve_pos_h1(t):
        waits = [(s_pe, job_idx[("P", t, 1)])]
        if t >= 2:
            waits.append((s_dve, 2 * (t - 2) + 1))
        attach = pre_waits(vec, waits)
        inst = vec.tensor_scalar_max(posb[:, t % 2], acc_pos.ap(), 0.0)
        inst._wait_ge(*attach)
        inst.then_inc(s_pos)
        inst = vec.tensor_scalar_max(hb[:, t % 2, NTILE:2 * NTILE],
                                     acc_h[:, NTILE:2 * NTILE], 0.0)
        inst._wait_ge(s_pe, job_idx[("H", t, 1)])
        inst.then_inc(s_h)

    def dve_prod_add(tp):
        r = tp % 2
        if tp < NT - 1:
            waits = [(s_vc, NVC[tp]), (s_pos, tp + 1)]
            if tp >= 1:
                waits.append((s_dve, 2 * tp))
            attach = pre_waits(vec, waits)
            inst = vec.tensor_mul(prodb.ap(), posb[:, r], vcb[:, r])
            inst._wait_ge(*attach)
            inst.then_inc(s_dve)
            waits = [(s_val, NVC[tp]), (s_dve, 2 * tp + 1)]
            if tp >= 2:
                waits.append((s_od[tp - 2], 16))
            attach = pre_waits(vec, waits)
            inst = vec.tensor_add(outt[:, r], valb[:, r], prodb.ap())
            inst._wait_ge(*attach)
            inst.then_inc(s_dve)
        else:
            # last tile: per-oc chains for a fast tail
            for oc in range(OC):
                sl = slice(oc * NTILE, (oc + 1) * NTILE)
                waits = [(s_vc, NVC[tp - 1] + oc + 1), (s_pos, tp + 1),
                         (s_dve, 2 * tp)]
                attach = pre_waits(vec, waits)
                inst = vec.tensor_mul(prodb[:, sl], posb[:, r, sl],
                                      vcb[:, r, sl])
                inst._wait_ge(*attach)
                inst.then_inc(s_dve)
            for oc in range(OC):
                sl = slice(oc * NTILE, (oc + 1) * NTILE)
                waits = [(s_val, NVC[tp - 1] + oc + 1),
                         (s_dve, 2 * tp + 2 + oc), (s_od[1], 16)]
                attach = pre_waits(vec, waits)
                inst = vec.tensor_add(outt[:, r, sl], valb[:, r, sl],
                                      prodb[:, sl])
                inst._wait_ge(*attach)
                inst.then_inc(s_dve)

    # ---- ACT program (availability order: h0(t+1) rides between vc(t)
    # and val(t) so W(t+1) is never starved on the h drains) --------------
    act_h0(0)
    act_vc(0)
    act_h0(1)
    act_val(0)
    act_h0(2)
    act_vc(1)
    act_val(1)
    act_h0(3)
    act_vc(2)
    act_val(2)
    act_vc(NT - 1)
    act_val(NT - 1)

    # ---- DVE program (availability order; pos/h drains lead so the PE
    # pipeline is never starved behind the prod/add chain) ----------------
    dve_pos_h1(0)
    dve_pos_h1(1)
    dve_prod_add(0)
    dve_pos_h1(2)
    dve_prod_add(1)
    dve_pos_h1(3)
    dve_prod_add(2)
    dve_prod_add(NT - 1)

    # ---- SP: output stores (issued off the DVE stream) ------------------
    for tp in range(NT - 1):
        inst = sp.dma_start(out=out_d.ap()[tp], in_=outt[:, tp % 2])
        inst._wait_ge(s_dve, 2 * (tp + 1))
        inst.then_inc(s_od[tp], 16)
    inst = sp.dma_start(out=out_d.ap()[NT - 1], in_=outt[:, (NT - 1) % 2])
    inst._wait_ge(s_dve, 2 * NT + 2)
    inst.then_inc(s_od[NT - 1], 16)

    # ---- epilogue: nothing. Engines fall straight into the compiler's
    # fixed epilogue (8-way barrier, then each engine wipes its fixed
    # slice of the semaphore space, then a final barrier+drain). The late
    # out-DMA sem increments may land after their sems are wiped, leaving
    # a +16 residue - harmless, nothing ever waits on s_od[2:], and the
    # next invocation's wipe clears it. s_od[0]/s_od[1] are last used
    # (waited) by DVE before its barrier arrival, so the wipe is ordered
    # after those uses by the barrier itself. ----------------------------
    nc.finalize()
    return nc


def _fold_scales(inputs):
    eps = np.float32(BN_EPS)
    s1 = np.asarray(inputs["bn1_g"], np.float32) / np.sqrt(
        np.asarray(inputs["bn1_v"], np.float32) + eps)
    b1 = np.asarray(inputs["bn1_b"], np.float32) - np.asarray(
        inputs["bn1_m"], np.float32) * s1
    s2 = np.asarray(inputs["bn2_g"], np.float32) / np.sqrt(
        np.asarray(inputs["bn2_v"], np.float32) + eps)
    b2 = np.asarray(inputs["bn2_b"], np.float32) - np.asarray(
        inputs["bn2_m"], np.float32) * s2
    return s1, b1, s2, b2


def _x_swizzle(inputs):
    f = np.asarray(inputs["features"], np.float32).astype(NPBF16)
    f2 = np.asarray(inputs["features2"], np.float32).astype(NPBF16)
    # xsw[core][it, p, s, kc, n] = x_s[core*NSH + it*NTILE + n, kc*P + p]
    xall = np.stack([f, f2], axis=0).reshape(
        2, NCORES, NT, NTILE, KC, P)          # s, core, it, n, kc, p
    xall = xall.transpose(1, 2, 5, 0, 4, 3)   # core, it, p, s, kc, n
    return np.ascontiguousarray(xall.reshape(NCORES, NT, P, 2 * KC * NTILE))


def _wsw_block(wt):  # [C, M] -> [KC, P, M] with row p = w(kc*P+p)
    m = wt.shape[1]
    return wt.reshape(KC, P, m)


def _prep_manual(inputs):
    xall = _x_swizzle(inputs)
    s1, b1, s2, b2 = _fold_scales(inputs)
    wkT = np.asarray(inputs["Wk"], np.float32).T
    wv1T = (np.asarray(inputs["Wv1"], np.float32) * s1[:, None]).T
    wv2T = (np.asarray(inputs["Wv2"], np.float32) * s2[:, None]).T
    wvcT = np.asarray(inputs["Wv_coef"], np.float32).T
    # wsw[p, j, kc, m]: j = 0:wk 1:wv1 2:wvc 3:wv2 (j-major so each
    # weight matrix is one contiguous 1KB/partition DMA)
    blocks = np.stack([_wsw_block(x) for x in (wkT, wv1T, wvcT, wv2T)],
                      axis=0)                   # j, kc, p, m
    wsw = np.ascontiguousarray(
        blocks.transpose(2, 0, 1, 3).reshape(P, 4 * KC * OUT).astype(NPBF16))
    in_maps = []
    for i in range(NCORES):
        in_maps.append({"xsw": xall[i], "wsw": wsw})
    return in_maps


def _unswizzle(osw):
    # osw [NT, P, OC*NTILE] bf16 -> [NSH, OUT] fp32
    o = np.asarray(osw).astype(np.float32).reshape(NT, P, OC, NTILE)
    return o.transpose(0, 3, 2, 1).reshape(NSH, OUT)


# --------------------------------------------------------------------------
# General fallback (nonzero folded bn biases): original TileContext kernel
# --------------------------------------------------------------------------

def _build_general():
    nc = bacc.Bacc(None, target_bir_lowering=False, debug=True)

    x_d = nc.declare_dram_parameter("xsw", [NT, P, 2 * KC * NTILE], BF16,
                                    isOutput=False)
    wk_d = nc.declare_dram_parameter("wksw", [P, KC * OUT], BF16,
                                     isOutput=False)
    w1_d = nc.declare_dram_parameter("w1sw", [P, KC * OUT], BF16,
                                     isOutput=False)
    wr_d = nc.declare_dram_parameter("wrsw", [P, KC * 2 * OUT], BF16,
                                     isOutput=False)
    sb_d = nc.declare_dram_parameter("sbsw", [P, OC * 2], F32, isOutput=False)
    out_d = nc.declare_dram_parameter("osw", [NT, P, OC * NTILE], BF16,
                                      isOutput=True)

    with tile.TileContext(nc) as tc:
        with (
            tc.tile_pool(name="wpool", bufs=1) as wpool,
            tc.tile_pool(name="inpool", bufs=NT) as inpool,
            tc.tile_pool(name="midpool", bufs=2) as midpool,
            tc.tile_pool(name="outpool", bufs=2) as outpool,
            tc.tile_pool(name="psum", bufs=1, space="PSUM") as psum,
        ):
            wk = wpool.tile([P, KC, OUT], BF16, tag="wk")
            nc.sync.dma_start(
                out=wk, in_=wk_d.ap().rearrange("p (kc o) -> p kc o", kc=KC))

            def load_x(it):
                t = inpool.tile([P, 2, KC, NTILE], BF16, tag="x")
                nc.sync.dma_start(
                    out=t,
                    in_=x_d.ap()[it].rearrange("p (s kc n) -> p s kc n",
                                               s=2, kc=KC))
                return t[:, 0], t[:, 1]

            xs = [load_x(0)]
            w1 = wpool.tile([P, KC, OUT], BF16, tag="w1")
            nc.sync.dma_start(
                out=w1, in_=w1_d.ap().rearrange("p (kc o) -> p kc o", kc=KC))
            sbt = wpool.tile([P, OC, 2], F32, tag="sbt")
            nc.sync.dma_start(
                out=sbt, in_=sb_d.ap().rearrange("p (oc c) -> p oc c", oc=OC))
            xs.append(load_x(1))
            wr = wpool.tile([P, KC, 2 * OUT], BF16, tag="wr")
            nc.sync.dma_start(
                out=wr, in_=wr_d.ap().rearrange("p (kc o) -> p kc o", kc=KC))
            for it in range(2, NT):
                xs.append(load_x(it))

            scratch = wpool.tile([P, NTILE], BF16, tag="scratch")
            nc.gpsimd.memset(scratch, 0.0)
            wacc = psum.tile([P, NTILE], F32, tag="acc_val1")
            for _ in range(8):
                nc.tensor.matmul(wacc, scratch[:, :P], scratch,
                                 start=True, stop=True)
            for _ in range(5):
                nc.tensor.matmul(wacc[:, :P], scratch[:, :P],
                                 scratch[:, :P], start=True, stop=True)
            dumm = wpool.tile([P, 1], F32, tag="dumm")
            nc.scalar.activation(dumm, scratch.bitcast(F32)[:, 0:1],
                                 AF.Sigmoid)

            def mm_group(w, woff, rhs, oc, tag):
                acc = psum.tile([P, NTILE], F32, tag=tag)
                for kc in range(KC):
                    nc.tensor.matmul(
                        acc,
                        w[:, kc, woff + oc * P:woff + (oc + 1) * P],
                        rhs[:, kc, :],
                        start=(kc == 0), stop=(kc == KC - 1))
                return acc

            def head(it):
                x1, x2 = xs[it]
                pos = midpool.tile([P, OC, NTILE], BF16, tag="pos")
                h = midpool.tile([P, OC, NTILE], BF16, tag="h")
                for oc in range(OC):
                    acc = mm_group(wk, 0, x1, oc, f"acc_pos{oc}")
                    nc.vector.tensor_scalar_max(pos[:, oc, :], acc, 0.0)
                for oc in range(OC):
                    acc = mm_group(w1, 0, x2, oc, f"acc_h{oc}")
                    if oc == 0:
                        nc.scalar.activation(h[:, oc, :], acc, AF.Relu,
                                             bias=sbt[:, oc, 0:1])
                    else:
                        nc.vector.tensor_scalar(h[:, oc, :], acc,
                                                sbt[:, oc, 0:1], 0.0,
                                                AluOpType.add, AluOpType.max)
                return it, pos, h

            def tail(state):
                it, pos, h = state
                vc = midpool.tile([P, OC, NTILE], BF16, tag="vc")
                prod = midpool.tile([P, OC, NTILE], BF16, tag="prod")
                val = midpool.tile([P, OC, NTILE], BF16, tag="val")
                outtt = outpool.tile([P, OC, NTILE], BF16, tag="outt")
                for oc in range(OC):
                    acc = mm_group(wr, 0, pos, oc, f"acc_vc{oc}")
                    nc.scalar.activation(vc[:, oc, :], acc, AF.Sigmoid)
                last = it == NT - 1
                osl = out_d.ap()[it].rearrange("p (oc n) -> p oc n", oc=OC)
                for oc in range(OC):
                    acc = mm_group(wr, OUT, h, oc, f"acc_val{oc}")
                    nc.vector.tensor_mul(prod[:, oc, :], pos[:, oc, :],
                                         vc[:, oc, :])
                    if last and oc == 0:
                        nc.vector.tensor_scalar(val[:, oc, :], acc,
                                                sbt[:, oc, 1:2], 0.0,
                                                AluOpType.add, AluOpType.max)
                    else:
                        nc.scalar.activation(val[:, oc, :], acc, AF.Relu,
                                             bias=sbt[:, oc, 1:2])
                    nc.vector.tensor_add(outtt[:, oc, :], val[:, oc, :],
                                         prod[:, oc, :])
                    if last:
                        nc.sync.dma_start(out=osl[:, oc], in_=outtt[:, oc])
                if not last:
                    nc.sync.dma_start(out=osl, in_=outtt)

            prev = None
            for it in range(NT):
                state = head(it)
                if prev is not None:
                    tail(prev)
                prev = state
            tail(prev)
    nc.finalize()
    return nc


def _prep_general(inputs):
    xall = _x_swizzle(inputs)
    s1, b1, s2, b2 = _fold_scales(inputs)
    wkT = np.asarray(inputs["Wk"], np.float32).T
    wv1T = (np.asarray(inputs["Wv1"], np.float32) * s1[:, None]).T
    wv2T = (np.asarray(inputs["Wv2"], np.float32) * s2[:, None]).T
    wvcT = np.asarray(inputs["Wv_coef"], np.float32).T

    def wsw(wt):
        m = wt.shape[1]
        return wt.reshape(KC, P, m).transpose(1, 0, 2).reshape(P, KC * m)

    wksw = np.ascontiguousarray(wsw(wkT).astype(NPBF16))
    w1sw = np.ascontiguousarray(wsw(wv1T).astype(NPBF16))
    wrT = np.concatenate([wvcT, wv2T], axis=1)
    wrsw = np.ascontiguousarray(wsw(wrT).astype(NPBF16))
    sbsw = np.ascontiguousarray(
        np.stack([b1, b2], axis=1).reshape(OC, P, 2)
        .transpose(1, 0, 2).reshape(P, OC * 2).astype(np.float32))

    in_maps = []
    for i in range(NCORES):
        in_maps.append({
            "xsw": xall[i],
            "wksw": wksw, "w1sw": w1sw, "wrsw": wrsw, "sbsw": sbsw,
        })
    return in_maps


# --------------------------------------------------------------------------

def _get_variant(inputs):
    _, b1, _, b2 = _fold_scales(inputs)
    if not np.any(b1) and not np.any(b2):
        return "manual"
    return "general"


def _run(inputs, trace=False, trace_cores=None, tmpdir=None):
    variant = _get_variant(inputs)
    if variant not in _cache:
        _cache[variant] = (_build_manual() if variant == "manual"
                           else _build_general())
    nc = _cache[variant]
    in_maps = (_prep_manual(inputs) if variant == "manual"
               else _prep_general(inputs))
    kw = {}
    if trace:
        kw = dict(trace=True, trace_cores=trace_cores or [0], tmpdir=tmpdir)
    res = run_bass_kernel_spmd(nc, in_maps, core_ids=list(range(NCORES)), **kw)
    out = np.empty((N_TOTAL, OUT), np.float32)
    for i in range(NCORES):
        out[i * NSH:(i + 1) * NSH, :] = _unswizzle(res.results[i]["osw"])
    return out, res


def kernel(**inputs):
    out, _ = _run(inputs, trace=False)
    return out
